# revision 57
# baseline (speedup 1.0000x reference)
"""Multi-head self-attention (RoPE, causal) Bass kernel for 8 TRN2 NeuronCores.

Problem: x (2, 2048, 1024) f32, wqkv (3072, 1024), wo (1024, 1024).
  qkv = x @ wqkv.T ; RoPE(q, k) ; causal softmax attention (16 heads, hd=64);
  out = y @ wo.T.

Sharding: batch (2-way) x head-group (4-way) tensor parallel = 8 cores.
Each core computes a full (2048, 1024) partial output for its batch from its
4 heads; host sums the 4 partials per batch (the TP all-reduce done at
unshard time).

v3 design notes (vs the 204us baseline):
- P@V runs as (V^T P): matmul(yT[65,512], lhsT=v[:,65H:65H+65], rhs=pt) so y
  lands directly in the transposed layout the wo projection needs. This kills
  the 544 tiny N=65 matmuls (LDWEIGHTS-bound), all 32 PE transposes and the
  y_all->yt copies. The ones-column in V makes psum row 64 the softmax
  denominator.
- softmax normalization: reciprocal_approx_fast (single DVE op) of yT row 64,
  broadcast across partitions with K=1 f32r PE matmuls, copy to sbuf, one
  [64,512] DVE mul per head writes normalized yt. The normalize for a
  head-pair is emitted one score-step into the NEXT head-pair so the PE
  never idles on it (and the yT bank reuse stays deadlock-free).
- causal diag masking via affine_select zeroing on pt (post-exp, Pool
  engine) instead of -1e9 PE matmuls; exp issued per-head on diag tiles to
  skip the dead columns.
- RoPE: cos-mul reads the fp16 sbuf copy (raw) so the qkv psum's only
  reader is the scalar copy; qkv m-chains interleave with v-proj chains so
  psum drains hide under PE work. PSUM plan (8 banks): ps 1 + psv 1 +
  big{scores,perm,bcast,wo} 4 + yT 2.
- OUT is fp16 (halves output DMA; host accumulates partials in fp32).

Precision: fp16 matmuls everywhere; exp and softmax accumulation in fp32.
"""
import sys

sys.path.insert(0, "/opt/trn_rl_repo")

import numpy as np

import concourse.bass as bass
import concourse.mybir as mybir
import concourse.tile as tile
from concourse import bacc, bass_utils

B, L, D = 2, 2048, 1024
NH, HD = 16, 64
NCORES = 8
HPC = 4            # heads per core
LQB = 512          # Lq block per S^T unit
NLQ = L // LQB     # 4
NLT = L // 128     # 16
KT = D // 128      # 8 contraction tiles for projections

F32 = mybir.dt.float32
F32R = mybir.dt.float32r
F16 = mybir.dt.float16
BF16 = mybir.dt.bfloat16
I16 = mybir.dt.int16

# Schraudolph fp16-bit exp: exp(s) ~= bitcast_f16(int16(EXP_A*s + EXP_B)).
# ~1.8% rms sawtooth error; applied to a third of the off-diagonal score
# tiles to take exp load off the Scalar engine (the attention bottleneck).
EXP_A = 1024.0 / float(np.log(2.0))
EXP_B = 15.0 * 1024.0 - 59.0

_cache = {}


def build_nc(debug=False):
    nc = bacc.Bacc("TRN2", target_bir_lowering=False, debug=False)

    XT = nc.dram_tensor("XT", [D, L], F16, kind="ExternalInput")
    WQKT = nc.dram_tensor("WQKT", [D, 512], F16, kind="ExternalInput")
    WVT = nc.dram_tensor("WVT", [D, 260], F16, kind="ExternalInput")
    WOT = nc.dram_tensor("WOT", [HPC * HD, D], F16, kind="ExternalInput")
    PERM = nc.dram_tensor("PERM", [128, 128], F16, kind="ExternalInput")
    COS = nc.dram_tensor("COS", [128, L], F16, kind="ExternalInput")
    SIN = nc.dram_tensor("SIN", [128, L], F32, kind="ExternalInput")
    SELR = nc.dram_tensor("SELR", [33, 128], F16, kind="ExternalInput")
    SUMZ = nc.dram_tensor("SUMZ", [33, 512], F16, kind="ExternalInput")
    OUT = nc.dram_tensor("OUT", [L, D], F16, kind="ExternalOutput")

    Exp = mybir.ActivationFunctionType.Exp

    with tile.TileContext(nc) as tc:
        with (
            tc.tile_pool(name="consts", bufs=1) as cpool,
            tc.tile_pool(name="weights", bufs=1) as wpool,
            tc.tile_pool(name="qkrot", bufs=1) as rotpool,
            tc.tile_pool(name="vsb", bufs=1) as vpool,
            tc.tile_pool(name="ytr", bufs=1) as ytpool,
            tc.tile_pool(name="xt", bufs=16) as xpool,
            tc.tile_pool(name="raws", bufs=3) as rawpool,
            tc.tile_pool(name="tmps", bufs=3) as tpool,
            tc.tile_pool(name="pts", bufs=4) as ptpool,
            tc.tile_pool(name="bcss", bufs=2) as bcspool,
            tc.tile_pool(name="outsb", bufs=3) as opool,
            tc.tile_pool(name="psP", bufs=1, space="PSUM") as pspool,
            tc.tile_pool(name="psV", bufs=1, space="PSUM") as vvpool,
            tc.tile_pool(name="psB", bufs=2, space="PSUM") as bigpool,
            tc.tile_pool(name="psY", bufs=1, space="PSUM") as ypool,
        ):
            # ---- static loads, ordered by first use so the PE can start
            # as soon as wqk + the first x chunk land ---------------------
            def load_xt_tile(j, k):
                xs = slice(j * LQB, (j + 1) * LQB)
                t = xpool.tile([128, LQB], F16, tag="xt", name="xt")
                if j == 0 and k == 0:
                    # first tile gates the first matmul: halve its latency
                    # by splitting across two DMA queues
                    nc.sync.dma_start(t[0:64, :], XT[0:64, xs])
                    nc.sync.dma_start(t[64:128, :], XT[64:128, xs])
                else:
                    nc.sync.dma_start(t[:], XT[k * 128:(k + 1) * 128, xs])
                return t

            def load_xt(j):
                return [load_xt_tile(j, k) for k in range(KT)]

            wqk_sb = []
            xt0 = []
            for k in range(KT):
                w = wpool.tile([128, 512], F16, tag=f"wqk{k}", name=f"wqk{k}")
                if k == 0:
                    nc.sync.dma_start(w[0:64, :], WQKT[0:64, :])
                    nc.sync.dma_start(w[64:128, :], WQKT[64:128, :])
                else:
                    nc.sync.dma_start(w[:], WQKT[k * 128:(k + 1) * 128, :])
                wqk_sb.append(w)
                xt0.append(load_xt_tile(0, k))

            wvt_sb = []
            for k in range(KT):
                wv = wpool.tile([128, 260], F16, tag=f"wv{k}", name=f"wv{k}")
                nc.sync.dma_start(wv[:], WVT[k * 128:(k + 1) * 128, :])
                wvt_sb.append(wv)
            cos_sb = cpool.tile([128, L], F16, tag="cos")
            nc.sync.dma_start(cos_sb[:], COS[:, :])
            sin_sb = cpool.tile([128, L], F32, tag="sin")
            nc.sync.dma_start(sin_sb[:], SIN[:, :])
            perm_sb = cpool.tile([128, 128], F16, tag="perm")
            nc.sync.dma_start(perm_sb[:], PERM[:, :])
            wot_sb = []
            for c2 in range(2):
                w = wpool.tile([128, D], F16, tag=f"wo{c2}", name=f"wo{c2}")
                nc.sync.dma_start(w[:], WOT[c2 * 128:(c2 + 1) * 128, :])
                wot_sb.append(w)
            # selector: one K=33 matmul broadcasts sums row 0 across output
            # partitions 0-63 and row 32 across 64-127
            sel_sb = cpool.tile([33, 128], F16, tag="sel")
            nc.sync.dma_start(sel_sb[:], SELR[:, :])
            # persistent rowsum row-pair tiles (rows 0 and 32 are written;
            # the zeroed rest keeps the K=33 matmul NaN-free)
            sums_sb = []
            for i in range(2):
                s = cpool.tile([33, 512], F16, tag=f"sums{i}")
                nc.sync.dma_start(s[:], SUMZ[:, :])
                sums_sb.append(s)

            # persistent activation storage
            # qk_rot[m]: m=0,1 -> q head-pairs (h01, h23); m=2,3 -> k pairs
            qk_rot = [rotpool.tile([128, L], F16, tag=f"rot{m}", name=f"rot{m}")
                      for m in range(4)]
            v_sb = [vpool.tile([128, 260], F16, tag=f"v{t}", name=f"v{t}")
                    for t in range(NLT)]
            # yt_sb[hp]: transposed, normalized y for head-pair hp
            yt_sb = [ytpool.tile([128, L], F16, tag=f"yt{c2}", name=f"yt{c2}")
                     for c2 in range(2)]

            def qkv_chunk(j, xt):
                xs = slice(j * LQB, (j + 1) * LQB)
                # q/k head-pair tiles with rope, interleaved with the v
                # tiles so each psum's drain hides under the next PE chain
                for m in range(4):
                    ps = pspool.tile([128, 512], F32, tag="ps", name="ps")
                    for k in range(KT):
                        nc.tensor.matmul(
                            ps[:], wqk_sb[k][:, m * 128:(m + 1) * 128],
                            xt[k][:],
                            start=(k == 0), stop=(k == KT - 1),
                        )
                    raw = rawpool.tile([128, LQB], F16, tag="raw")
                    nc.scalar.copy(raw[:], ps[:])
                    # t1 reads the sbuf fp16 copy, so ps's only reader is
                    # the scalar copy — the next chain isn't gated on DVE
                    t1 = tpool.tile([128, LQB], F16, tag="t1")
                    nc.vector.tensor_mul(t1[:], raw[:], cos_sb[:, xs])
                    # v tile (natural L x hd layout, ones col after each
                    # head) — emitted between the m-chain and the perm
                    # matmul so the PE never head-blocks on the raw copy
                    ti = j * 4 + m
                    psv = vvpool.tile([128, 512], F32, tag="vv", name="vv")
                    for k in range(KT):
                        nc.tensor.matmul(
                            psv[:, 0:260], xt[k][:, m * 128:(m + 1) * 128],
                            wvt_sb[k][:],
                            start=(k == 0), stop=(k == KT - 1),
                        )
                    nc.scalar.copy(v_sb[ti][:], psv[:, 0:260])
                    nc.vector.memset(v_sb[ti][:, 64:260:65], 1.0)
                    pswt = bigpool.tile([128, 1024], F32, tag="big",
                                        name="psw")
                    psw = pswt[:, 0:512]
                    nc.tensor.matmul(psw, perm_sb[:], raw[:],
                                     start=True, stop=True)
                    t2 = tpool.tile([128, LQB], F16, tag="t2")
                    nc.vector.tensor_mul(t2[:], psw, sin_sb[:, xs])
                    nc.vector.tensor_add(qk_rot[m][:, xs], t1[:], t2[:])

            wo_ready = []

            def wo_tile(i):
                po = bigpool.tile([128, 1024], F32, tag="big", name="po")
                for half in range(2):
                    for c2 in range(2):
                        nc.tensor.matmul(
                            po[:, 512 * half:512 * half + 512],
                            yt_sb[c2][:, 128 * i:128 * i + 128],
                            wot_sb[c2][:, 512 * half:512 * half + 512],
                            start=(c2 == 0), stop=(c2 == 1),
                        )
                ob = opool.tile([128, 1024], F16, tag="ob")
                # split halves across engines: lower latency than either
                # engine doing the whole copy, so the psum bank frees fast
                nc.scalar.copy(ob[:, 0:512], po[:, 0:512])
                nc.vector.tensor_copy(ob[:, 512:1024], po[:, 512:1024])
                nc.gpsimd.dma_start(OUT[128 * i:128 * i + 128, :], ob[:])

            def emit_wo(nmax):
                for _ in range(nmax):
                    if not wo_ready:
                        return
                    wo_tile(wo_ready.pop(0))

            def emit_norm(pending):
                """PE broadcast + sbuf copy + DVE muls for a finished
                head-pair; emitted behind other PE work so it never
                head-blocks the queue. Once a jq's second head-pair is
                normalized, its wo tiles become emittable."""
                hp, jq, yT, sums = pending
                # broadcast both heads' rowsums across partitions with one
                # K=33 matmul, then one approx-reciprocal covers both heads
                bc = vvpool.tile([128, 512], F32, tag="vv", name="bc")
                nc.tensor.matmul(bc[:], sel_sb[:], sums[:],
                                 start=True, stop=True)
                bcs = bcspool.tile([128, 512], F32, tag="bcs")
                nc.vector.reciprocal_approx_fast(bcs[:], bc[:])
                for h in range(2):
                    nc.vector.tensor_mul(
                        yt_sb[hp][64 * h:64 * h + 64,
                                  jq * LQB:(jq + 1) * LQB],
                        yT[h][0:64, :],
                        bcs[64 * h:64 * h + 64, :],
                    )
                if hp == 1:
                    wo_ready.extend(range(4 * jq, 4 * jq + 4))

            pending_norm = [None]

            def attention_jq(jq):
                nt = 4 * jq + 4  # causal: Lk tiles 0 .. 4jq+3
                for hp in range(2):
                    if hp == 1:
                        # fill the head-pair seam (PE waits on the previous
                        # pair's normalize chain anyway) with ready wo tiles
                        emit_wo(2 if jq < 3 else 4)
                    # each yT tile owns one PSUM bank; its t=0 matmul covers
                    # the full [0:512] width, so start=True zeroes the bank
                    # (no sibling chains share it)
                    yT = [ypool.tile([65, 512], F32, tag=f"yT{h}",
                                     name=f"yT{h}", bufs=1)
                          for h in range(2)]

                    def pv_th(t, pt, h):
                        off = max(0, t * 128 - jq * LQB)
                        H = 2 * hp + h
                        nc.tensor.matmul(
                            yT[h][:, off:512],
                            v_sb[t][:, 65 * H:65 * H + 65],
                            pt[:, 512 * h + off:512 * h + 512],
                            start=(t == 0), stop=(t == nt - 1),
                            skip_group_check=True,
                        )

                    prev = None
                    for t in range(nt):
                        diag = t >= 4 * jq
                        # causal trim: cols < off are fully masked
                        off = max(0, t * 128 - jq * LQB)
                        sp = bigpool.tile([128, 1024], F32, tag="big",
                                          name="sp")
                        # interleave scores with the lagged P@V so every
                        # LDWEIGHTS prefetches under the previous matmul
                        for h in range(2):
                            hs = slice(64 * h, 64 * h + 64)
                            nc.tensor.matmul(
                                sp[:, 512 * h + off:512 * h + 512],
                                qk_rot[2 + hp][hs, t * 128:(t + 1) * 128],
                                qk_rot[hp][hs, jq * LQB + off:
                                           (jq + 1) * LQB],
                                start=True, stop=True,
                            )
                            if prev is not None:
                                pv_th(*prev, h)
                        if t == 0 and pending_norm[0] is not None:
                            emit_norm(pending_norm[0])
                            pending_norm[0] = None
                        pt = ptpool.tile([128, 1024], F16, tag="pt")
                        if diag:
                            for h in range(2):
                                nc.scalar.activation(
                                    pt[:, 512 * h + off:512 * h + 512],
                                    sp[:, 512 * h + off:512 * h + 512],
                                    Exp,
                                )
                            # zero the upper triangle of the diag stripe
                            # (key > query) so P@V sees true zeros
                            for h in range(2):
                                nc.gpsimd.affine_select(
                                    out=pt[:, 512 * h + off:
                                           512 * h + off + 128],
                                    in_=pt[:, 512 * h + off:
                                           512 * h + off + 128],
                                    compare_op=mybir.AluOpType.is_ge,
                                    fill=0.0,
                                    base=0,
                                    pattern=[[1, 128]],
                                    channel_multiplier=-1,
                                )
                        elif t % 3 == 1:
                            # approximate exp on the DVE to unload Scalar
                            nc.vector.tensor_scalar(
                                pt[:, 0:1024].bitcast(I16),
                                sp[:, 0:1024],
                                EXP_A, EXP_B,
                                mybir.AluOpType.mult, mybir.AluOpType.add,
                            )
                        else:
                            nc.scalar.activation(pt[:, 0:1024], sp[:, 0:1024],
                                                 Exp)
                        prev = (t, pt)
                    pv_th(*prev, 0)
                    pv_th(*prev, 1)
                    # rowsums to sbuf right away (split engines so both
                    # copies run in parallel); the rest of the normalize
                    # is deferred into the next PE block
                    sums = sums_sb[hp]
                    nc.scalar.copy(sums[0:1, :], yT[0][64:65, :])
                    nc.vector.tensor_copy(sums[32:33, :], yT[1][64:65, :])
                    if pending_norm[0] is not None:
                        emit_norm(pending_norm[0])
                    pending_norm[0] = (hp, jq, yT, sums)

            # software pipeline: qkv runs one chunk ahead of attention so
            # the PE never waits on rope at the seams; wo tiles trail,
            # filling head-pair seams and chunk boundaries
            qkv_chunk(0, xt0)
            attention_jq(0)
            qkv_chunk(1, load_xt(1))
            attention_jq(1)
            qkv_chunk(2, load_xt(2))
            emit_wo(2)
            attention_jq(2)
            qkv_chunk(3, load_xt(3))
            emit_wo(2)
            attention_jq(3)
            emit_norm(pending_norm[0])
            pending_norm[0] = None
            emit_wo(16)

    nc.finalize()
    return nc


def prep_inputs(x, wqkv, wo):
    """Build the 8 per-core input dicts from the full-problem inputs."""
    x = np.asarray(x, dtype=np.float32)
    wqkv = np.asarray(wqkv, dtype=np.float32)
    wo = np.asarray(wo, dtype=np.float32)

    # rope tables
    inv_freq = 1.0 / (10000.0 ** (np.arange(0, HD, 2, dtype=np.float32) / HD))
    t = np.arange(L, dtype=np.float32)
    freqs = np.outer(t, inv_freq)                  # (L, 32)
    cos32 = np.cos(freqs).T.astype(np.float32)     # (32, L)
    sin32 = np.sin(freqs).T.astype(np.float32)
    COS = np.ascontiguousarray(np.tile(cos32, (4, 1)))           # (128, L)
    SIN = np.ascontiguousarray(
        np.concatenate([-sin32, sin32, -sin32, sin32], axis=0)
    )

    # 32-block swap permutation (within each head's 64 rows)
    PERM = np.zeros((128, 128), dtype=np.float32)
    for blk in range(2):
        o = 64 * blk
        PERM[o:o + 32, o + 32:o + 64] = np.eye(32)
        PERM[o + 32:o + 64, o:o + 32] = np.eye(32)

    # rowsum-broadcast selector: row 0 -> out partitions 0-63,
    # row 32 -> out partitions 64-127
    SEL = np.zeros((33, 128), dtype=np.float32)
    SEL[0, 0:64] = 1.0
    SEL[32, 64:128] = 1.0

    in_maps = []
    scale = np.float32(HD ** -0.5)
    for c in range(NCORES):
        b, g = divmod(c, 4)
        qrows = slice(256 * g, 256 * g + 256)
        krows = slice(1024 + 256 * g, 1024 + 256 * g + 256)
        vrows = slice(2048 + 256 * g, 2048 + 256 * g + 256)

        XT = np.ascontiguousarray(x[b].T)                        # (1024, 2048)
        wq = (wqkv[qrows, :] * scale).T                          # (1024, 256)
        wk = wqkv[krows, :].T
        WQKT = np.ascontiguousarray(np.concatenate([wq, wk], axis=1))
        vpart = wqkv[vrows, :].T                                 # (1024, 256)
        WVT = np.zeros((D, 260), dtype=np.float32)
        for h in range(HPC):
            WVT[:, 65 * h:65 * h + 64] = vpart[:, 64 * h:64 * h + 64]
        WOT = np.ascontiguousarray(wo[:, 256 * g:256 * g + 256].T)

        in_maps.append({
            "XT": XT.astype(np.float16),
            "WQKT": WQKT.astype(np.float16),
            "WVT": WVT.astype(np.float16),
            "WOT": WOT.astype(np.float16),
            "COS": COS.astype(np.float16),
            "SIN": SIN,
            "PERM": PERM.astype(np.float16),
            "SELR": SEL.astype(np.float16),
            "SUMZ": np.zeros((33, 512), dtype=np.float16),
        })
    return in_maps


def kernel(x, wqkv, wo):
    if "nc" not in _cache:
        _cache["nc"] = build_nc()
    nc = _cache["nc"]
    in_maps = prep_inputs(x, wqkv, wo)
    res = bass_utils.run_bass_kernel_spmd(nc, in_maps, list(range(NCORES)))
    outs = [res.results[c]["OUT"].astype(np.float32) for c in range(NCORES)]
    out0 = outs[0] + outs[1] + outs[2] + outs[3]
    out1 = outs[4] + outs[5] + outs[6] + outs[7]
    return np.stack([out0, out1]).astype(np.float32)


# revision 58
# speedup vs baseline: 1.0174x; 1.0174x over previous
"""Multi-head self-attention (RoPE, causal) Bass kernel for 8 TRN2 NeuronCores.

Problem: x (2, 2048, 1024) f32, wqkv (3072, 1024), wo (1024, 1024).
  qkv = x @ wqkv.T ; RoPE(q, k) ; causal softmax attention (16 heads, hd=64);
  out = y @ wo.T.

Sharding: batch (2-way) x head-group (4-way) tensor parallel = 8 cores.
Each core computes a full (2048, 1024) partial output for its batch from its
4 heads; host sums the 4 partials per batch (the TP all-reduce done at
unshard time).

v3 design notes (vs the 204us baseline):
- P@V runs as (V^T P): matmul(yT[65,512], lhsT=v[:,65H:65H+65], rhs=pt) so y
  lands directly in the transposed layout the wo projection needs. This kills
  the 544 tiny N=65 matmuls (LDWEIGHTS-bound), all 32 PE transposes and the
  y_all->yt copies. The ones-column in V makes psum row 64 the softmax
  denominator.
- softmax normalization: reciprocal_approx_fast (single DVE op) of yT row 64,
  broadcast across partitions with K=1 f32r PE matmuls, copy to sbuf, one
  [64,512] DVE mul per head writes normalized yt. The normalize for a
  head-pair is emitted one score-step into the NEXT head-pair so the PE
  never idles on it (and the yT bank reuse stays deadlock-free).
- causal diag masking via affine_select zeroing on pt (post-exp, Pool
  engine) instead of -1e9 PE matmuls; exp issued per-head on diag tiles to
  skip the dead columns.
- RoPE: cos-mul reads the fp16 sbuf copy (raw) so the qkv psum's only
  reader is the scalar copy; qkv m-chains interleave with v-proj chains so
  psum drains hide under PE work. PSUM plan (8 banks): ps 1 + psv 1 +
  big{scores,perm,bcast,wo} 4 + yT 2.
- OUT is fp16 (halves output DMA; host accumulates partials in fp32).

Precision: fp16 matmuls everywhere; exp and softmax accumulation in fp32.
"""
import sys

sys.path.insert(0, "/opt/trn_rl_repo")

import numpy as np

import concourse.bass as bass
import concourse.mybir as mybir
import concourse.tile as tile
from concourse import bacc, bass_utils

B, L, D = 2, 2048, 1024
NH, HD = 16, 64
NCORES = 8
HPC = 4            # heads per core
LQB = 512          # Lq block per S^T unit
NLQ = L // LQB     # 4
NLT = L // 128     # 16
KT = D // 128      # 8 contraction tiles for projections

F32 = mybir.dt.float32
F32R = mybir.dt.float32r
F16 = mybir.dt.float16
BF16 = mybir.dt.bfloat16
I16 = mybir.dt.int16

# Schraudolph fp16-bit exp: exp(s) ~= bitcast_f16(int16(EXP_A*s + EXP_B)).
# ~1.8% rms sawtooth error; applied to a third of the off-diagonal score
# tiles to take exp load off the Scalar engine (the attention bottleneck).
EXP_A = 1024.0 / float(np.log(2.0))
EXP_B = 15.0 * 1024.0 - 59.0

_cache = {}


def build_nc(debug=False):
    nc = bacc.Bacc("TRN2", target_bir_lowering=False, debug=False)

    XT = nc.dram_tensor("XT", [D, L], F16, kind="ExternalInput")
    WQKT = nc.dram_tensor("WQKT", [D, 512], F16, kind="ExternalInput")
    WVT = nc.dram_tensor("WVT", [D, 260], F16, kind="ExternalInput")
    WOT = nc.dram_tensor("WOT", [HPC * HD, D], F16, kind="ExternalInput")
    PERM = nc.dram_tensor("PERM", [128, 128], F16, kind="ExternalInput")
    COS = nc.dram_tensor("COS", [128, L], F16, kind="ExternalInput")
    SIN = nc.dram_tensor("SIN", [128, L], F32, kind="ExternalInput")
    SELR = nc.dram_tensor("SELR", [33, 128], F16, kind="ExternalInput")
    SUMZ = nc.dram_tensor("SUMZ", [33, 512], F16, kind="ExternalInput")
    OUT = nc.dram_tensor("OUT", [L, D], F16, kind="ExternalOutput")

    Exp = mybir.ActivationFunctionType.Exp

    with tile.TileContext(nc) as tc:
        with (
            tc.tile_pool(name="consts", bufs=1) as cpool,
            tc.tile_pool(name="weights", bufs=1) as wpool,
            tc.tile_pool(name="qkrot", bufs=1) as rotpool,
            tc.tile_pool(name="vsb", bufs=1) as vpool,
            tc.tile_pool(name="ytr", bufs=1) as ytpool,
            tc.tile_pool(name="xt", bufs=16) as xpool,
            tc.tile_pool(name="raws", bufs=3) as rawpool,
            tc.tile_pool(name="tmps", bufs=3) as tpool,
            tc.tile_pool(name="pts", bufs=4) as ptpool,
            tc.tile_pool(name="bcss", bufs=2) as bcspool,
            tc.tile_pool(name="outsb", bufs=3) as opool,
            tc.tile_pool(name="psP", bufs=1, space="PSUM") as pspool,
            tc.tile_pool(name="psV", bufs=1, space="PSUM") as vvpool,
            tc.tile_pool(name="psB", bufs=2, space="PSUM") as bigpool,
            tc.tile_pool(name="psY", bufs=1, space="PSUM") as ypool,
        ):
            # ---- static loads, ordered by first use so the PE can start
            # as soon as wqk + the first x chunk land ---------------------
            def load_xt_tile(j, k):
                xs = slice(j * LQB, (j + 1) * LQB)
                t = xpool.tile([128, LQB], F16, tag="xt", name="xt")
                if j == 0 and k == 0:
                    # first tile gates the first matmul: halve its latency
                    # by splitting across two DMA queues
                    nc.sync.dma_start(t[0:64, :], XT[0:64, xs])
                    nc.sync.dma_start(t[64:128, :], XT[64:128, xs])
                else:
                    nc.sync.dma_start(t[:], XT[k * 128:(k + 1) * 128, xs])
                return t

            def load_xt(j):
                return [load_xt_tile(j, k) for k in range(KT)]

            wqk_sb = []
            xt0 = []
            for k in range(KT):
                w = wpool.tile([128, 512], F16, tag=f"wqk{k}", name=f"wqk{k}")
                if k == 0:
                    nc.sync.dma_start(w[0:64, :], WQKT[0:64, :])
                    nc.sync.dma_start(w[64:128, :], WQKT[64:128, :])
                else:
                    nc.sync.dma_start(w[:], WQKT[k * 128:(k + 1) * 128, :])
                wqk_sb.append(w)
                xt0.append(load_xt_tile(0, k))

            wvt_sb = []
            for k in range(KT):
                wv = wpool.tile([128, 260], F16, tag=f"wv{k}", name=f"wv{k}")
                nc.sync.dma_start(wv[:], WVT[k * 128:(k + 1) * 128, :])
                wvt_sb.append(wv)
            cos_sb = cpool.tile([128, L], F16, tag="cos")
            nc.sync.dma_start(cos_sb[:], COS[:, :])
            sin_sb = cpool.tile([128, L], F32, tag="sin")
            nc.sync.dma_start(sin_sb[:], SIN[:, :])
            perm_sb = cpool.tile([128, 128], F16, tag="perm")
            nc.sync.dma_start(perm_sb[:], PERM[:, :])
            wot_sb = []
            for c2 in range(2):
                w = wpool.tile([128, D], F16, tag=f"wo{c2}", name=f"wo{c2}")
                nc.sync.dma_start(w[:], WOT[c2 * 128:(c2 + 1) * 128, :])
                wot_sb.append(w)
            # selector: one K=33 matmul broadcasts sums row 0 across output
            # partitions 0-63 and row 32 across 64-127
            sel_sb = cpool.tile([33, 128], F16, tag="sel")
            nc.sync.dma_start(sel_sb[:], SELR[:, :])
            # persistent rowsum row-pair tiles (rows 0 and 32 are written;
            # the zeroed rest keeps the K=33 matmul NaN-free)
            sums_sb = []
            for i in range(2):
                s = cpool.tile([33, 512], F16, tag=f"sums{i}")
                nc.sync.dma_start(s[:], SUMZ[:, :])
                sums_sb.append(s)

            # persistent activation storage
            # qk_rot[m]: m=0,1 -> q head-pairs (h01, h23); m=2,3 -> k pairs
            qk_rot = [rotpool.tile([128, L], F16, tag=f"rot{m}", name=f"rot{m}")
                      for m in range(4)]
            v_sb = [vpool.tile([128, 260], F16, tag=f"v{t}", name=f"v{t}")
                    for t in range(NLT)]
            # yt_sb[hp]: transposed, normalized y for head-pair hp
            yt_sb = [ytpool.tile([128, L], F16, tag=f"yt{c2}", name=f"yt{c2}")
                     for c2 in range(2)]

            def qkv_chunk(j, xt):
                xs = slice(j * LQB, (j + 1) * LQB)
                # q/k head-pair tiles with rope, interleaved with the v
                # tiles so each psum's drain hides under the next PE chain
                for m in range(4):
                    ps = pspool.tile([128, 512], F32, tag="ps", name="ps")
                    for k in range(KT):
                        nc.tensor.matmul(
                            ps[:], wqk_sb[k][:, m * 128:(m + 1) * 128],
                            xt[k][:],
                            start=(k == 0), stop=(k == KT - 1),
                        )
                    raw = rawpool.tile([128, LQB], F16, tag="raw")
                    nc.scalar.copy(raw[:], ps[:])
                    # t1 reads the sbuf fp16 copy, so ps's only reader is
                    # the scalar copy — the next chain isn't gated on DVE
                    t1 = tpool.tile([128, LQB], F16, tag="t1")
                    nc.vector.tensor_mul(t1[:], raw[:], cos_sb[:, xs])
                    # v tile (natural L x hd layout, ones col after each
                    # head) — emitted between the m-chain and the perm
                    # matmul so the PE never head-blocks on the raw copy
                    ti = j * 4 + m
                    psv = vvpool.tile([128, 512], F32, tag="vv", name="vv")
                    for k in range(KT):
                        nc.tensor.matmul(
                            psv[:, 0:260], xt[k][:, m * 128:(m + 1) * 128],
                            wvt_sb[k][:],
                            start=(k == 0), stop=(k == KT - 1),
                        )
                    nc.scalar.copy(v_sb[ti][:], psv[:, 0:260])
                    nc.vector.memset(v_sb[ti][:, 64:260:65], 1.0)
                    pswt = bigpool.tile([128, 1024], F32, tag="big",
                                        name="psw")
                    psw = pswt[:, 0:512]
                    nc.tensor.matmul(psw, perm_sb[:], raw[:],
                                     start=True, stop=True)
                    t2 = tpool.tile([128, LQB], F16, tag="t2")
                    nc.vector.tensor_mul(t2[:], psw, sin_sb[:, xs])
                    nc.vector.tensor_add(qk_rot[m][:, xs], t1[:], t2[:])

            wo_ready = []

            def wo_tile(i):
                po = bigpool.tile([128, 1024], F32, tag="big", name="po")
                for half in range(2):
                    for c2 in range(2):
                        nc.tensor.matmul(
                            po[:, 512 * half:512 * half + 512],
                            yt_sb[c2][:, 128 * i:128 * i + 128],
                            wot_sb[c2][:, 512 * half:512 * half + 512],
                            start=(c2 == 0), stop=(c2 == 1),
                        )
                ob = opool.tile([128, 1024], F16, tag="ob")
                # split halves across engines: lower latency than either
                # engine doing the whole copy, so the psum bank frees fast
                nc.scalar.copy(ob[:, 0:512], po[:, 0:512])
                nc.vector.tensor_copy(ob[:, 512:1024], po[:, 512:1024])
                nc.gpsimd.dma_start(OUT[128 * i:128 * i + 128, :], ob[:])

            def emit_wo(nmax):
                for _ in range(nmax):
                    if not wo_ready:
                        return
                    wo_tile(wo_ready.pop(0))

            def emit_norm(pending):
                """PE broadcast + sbuf copy + DVE muls for a finished
                head-pair; emitted behind other PE work so it never
                head-blocks the queue. Once a jq's second head-pair is
                normalized, its wo tiles become emittable."""
                hp, jq, yT, sums = pending
                # broadcast both heads' rowsums across partitions with one
                # K=33 matmul, then one approx-reciprocal covers both heads
                bc = vvpool.tile([128, 512], F32, tag="vv", name="bc")
                nc.tensor.matmul(bc[:], sel_sb[:], sums[:],
                                 start=True, stop=True)
                bcs = bcspool.tile([128, 512], F32, tag="bcs")
                nc.vector.reciprocal_approx_fast(bcs[:], bc[:])
                for h in range(2):
                    nc.vector.tensor_mul(
                        yt_sb[hp][64 * h:64 * h + 64,
                                  jq * LQB:(jq + 1) * LQB],
                        yT[h][0:64, :],
                        bcs[64 * h:64 * h + 64, :],
                    )
                if hp == 1:
                    wo_ready.extend(range(4 * jq, 4 * jq + 4))

            pending_norm = [None]

            def attention_jq(jq):
                nt = 4 * jq + 4  # causal: Lk tiles 0 .. 4jq+3
                for hp in range(2):
                    if hp == 1:
                        # fill the head-pair seam (PE waits on the previous
                        # pair's normalize chain anyway) with ready wo tiles
                        emit_wo(2 if jq < 3 else 4)
                    # each yT tile owns one PSUM bank; its t=0 matmul covers
                    # the full [0:512] width, so start=True zeroes the bank
                    # (no sibling chains share it)
                    yT = [ypool.tile([65, 512], F32, tag=f"yT{h}",
                                     name=f"yT{h}", bufs=1)
                          for h in range(2)]

                    def pv_th(t, pt, h):
                        off = max(0, t * 128 - jq * LQB)
                        H = 2 * hp + h
                        nc.tensor.matmul(
                            yT[h][:, off:512],
                            v_sb[t][:, 65 * H:65 * H + 65],
                            pt[:, 512 * h + off:512 * h + 512],
                            start=(t == 0), stop=(t == nt - 1),
                            skip_group_check=True,
                        )

                    prev = None
                    for t in range(nt):
                        diag = t >= 4 * jq
                        # causal trim: cols < off are fully masked
                        off = max(0, t * 128 - jq * LQB)
                        sp = bigpool.tile([128, 1024], F32, tag="big",
                                          name="sp")
                        for h in range(2):
                            hs = slice(64 * h, 64 * h + 64)
                            nc.tensor.matmul(
                                sp[:, 512 * h + off:512 * h + 512],
                                qk_rot[2 + hp][hs, t * 128:(t + 1) * 128],
                                qk_rot[hp][hs, jq * LQB + off:
                                           (jq + 1) * LQB],
                                start=True, stop=True,
                            )
                        # P@V lags the scores by one tile so the exp
                        # latency hides behind the next score matmuls
                        if prev is not None:
                            pv_th(*prev, 0)
                            pv_th(*prev, 1)
                        if t == 0 and pending_norm[0] is not None:
                            emit_norm(pending_norm[0])
                            pending_norm[0] = None
                        pt = ptpool.tile([128, 1024], F16, tag="pt")
                        if diag:
                            for h in range(2):
                                nc.scalar.activation(
                                    pt[:, 512 * h + off:512 * h + 512],
                                    sp[:, 512 * h + off:512 * h + 512],
                                    Exp,
                                )
                            # zero the upper triangle of the diag stripe
                            # (key > query) so P@V sees true zeros
                            for h in range(2):
                                nc.gpsimd.affine_select(
                                    out=pt[:, 512 * h + off:
                                           512 * h + off + 128],
                                    in_=pt[:, 512 * h + off:
                                           512 * h + off + 128],
                                    compare_op=mybir.AluOpType.is_ge,
                                    fill=0.0,
                                    base=0,
                                    pattern=[[1, 128]],
                                    channel_multiplier=-1,
                                )
                        elif t % 3 == 1:
                            # approximate exp on the DVE to unload Scalar
                            nc.vector.tensor_scalar(
                                pt[:, 0:1024].bitcast(I16),
                                sp[:, 0:1024],
                                EXP_A, EXP_B,
                                mybir.AluOpType.mult, mybir.AluOpType.add,
                            )
                        else:
                            nc.scalar.activation(pt[:, 0:1024], sp[:, 0:1024],
                                                 Exp)
                        prev = (t, pt)
                    pv_th(*prev, 0)
                    pv_th(*prev, 1)
                    # rowsums to sbuf right away (split engines so both
                    # copies run in parallel); the rest of the normalize
                    # is deferred into the next PE block
                    sums = sums_sb[hp]
                    nc.scalar.copy(sums[0:1, :], yT[0][64:65, :])
                    nc.vector.tensor_copy(sums[32:33, :], yT[1][64:65, :])
                    if pending_norm[0] is not None:
                        emit_norm(pending_norm[0])
                    pending_norm[0] = (hp, jq, yT, sums)

            # software pipeline: qkv runs one chunk ahead of attention so
            # the PE never waits on rope at the seams; wo tiles trail,
            # filling head-pair seams and chunk boundaries
            qkv_chunk(0, xt0)
            attention_jq(0)
            qkv_chunk(1, load_xt(1))
            attention_jq(1)
            qkv_chunk(2, load_xt(2))
            emit_wo(2)
            attention_jq(2)
            qkv_chunk(3, load_xt(3))
            emit_wo(2)
            attention_jq(3)
            emit_norm(pending_norm[0])
            pending_norm[0] = None
            emit_wo(16)

    nc.finalize()
    return nc


def prep_inputs(x, wqkv, wo):
    """Build the 8 per-core input dicts from the full-problem inputs."""
    x = np.asarray(x, dtype=np.float32)
    wqkv = np.asarray(wqkv, dtype=np.float32)
    wo = np.asarray(wo, dtype=np.float32)

    # rope tables
    inv_freq = 1.0 / (10000.0 ** (np.arange(0, HD, 2, dtype=np.float32) / HD))
    t = np.arange(L, dtype=np.float32)
    freqs = np.outer(t, inv_freq)                  # (L, 32)
    cos32 = np.cos(freqs).T.astype(np.float32)     # (32, L)
    sin32 = np.sin(freqs).T.astype(np.float32)
    COS = np.ascontiguousarray(np.tile(cos32, (4, 1)))           # (128, L)
    SIN = np.ascontiguousarray(
        np.concatenate([-sin32, sin32, -sin32, sin32], axis=0)
    )

    # 32-block swap permutation (within each head's 64 rows)
    PERM = np.zeros((128, 128), dtype=np.float32)
    for blk in range(2):
        o = 64 * blk
        PERM[o:o + 32, o + 32:o + 64] = np.eye(32)
        PERM[o + 32:o + 64, o:o + 32] = np.eye(32)

    # rowsum-broadcast selector: row 0 -> out partitions 0-63,
    # row 32 -> out partitions 64-127
    SEL = np.zeros((33, 128), dtype=np.float32)
    SEL[0, 0:64] = 1.0
    SEL[32, 64:128] = 1.0

    in_maps = []
    scale = np.float32(HD ** -0.5)
    for c in range(NCORES):
        b, g = divmod(c, 4)
        qrows = slice(256 * g, 256 * g + 256)
        krows = slice(1024 + 256 * g, 1024 + 256 * g + 256)
        vrows = slice(2048 + 256 * g, 2048 + 256 * g + 256)

        XT = np.ascontiguousarray(x[b].T)                        # (1024, 2048)
        wq = (wqkv[qrows, :] * scale).T                          # (1024, 256)
        wk = wqkv[krows, :].T
        WQKT = np.ascontiguousarray(np.concatenate([wq, wk], axis=1))
        vpart = wqkv[vrows, :].T                                 # (1024, 256)
        WVT = np.zeros((D, 260), dtype=np.float32)
        for h in range(HPC):
            WVT[:, 65 * h:65 * h + 64] = vpart[:, 64 * h:64 * h + 64]
        WOT = np.ascontiguousarray(wo[:, 256 * g:256 * g + 256].T)

        in_maps.append({
            "XT": XT.astype(np.float16),
            "WQKT": WQKT.astype(np.float16),
            "WVT": WVT.astype(np.float16),
            "WOT": WOT.astype(np.float16),
            "COS": COS.astype(np.float16),
            "SIN": SIN,
            "PERM": PERM.astype(np.float16),
            "SELR": SEL.astype(np.float16),
            "SUMZ": np.zeros((33, 512), dtype=np.float16),
        })
    return in_maps


def kernel(x, wqkv, wo):
    if "nc" not in _cache:
        _cache["nc"] = build_nc()
    nc = _cache["nc"]
    in_maps = prep_inputs(x, wqkv, wo)
    res = bass_utils.run_bass_kernel_spmd(nc, in_maps, list(range(NCORES)))
    outs = [res.results[c]["OUT"].astype(np.float32) for c in range(NCORES)]
    out0 = outs[0] + outs[1] + outs[2] + outs[3]
    out1 = outs[4] + outs[5] + outs[6] + outs[7]
    return np.stack([out0, out1]).astype(np.float32)


# revision 59
# speedup vs baseline: 1.0215x; 1.0040x over previous
"""Multi-head self-attention (RoPE, causal) Bass kernel for 8 TRN2 NeuronCores.

Problem: x (2, 2048, 1024) f32, wqkv (3072, 1024), wo (1024, 1024).
  qkv = x @ wqkv.T ; RoPE(q, k) ; causal softmax attention (16 heads, hd=64);
  out = y @ wo.T.

Sharding: batch (2-way) x head-group (4-way) tensor parallel = 8 cores.
Each core computes a full (2048, 1024) partial output for its batch from its
4 heads; host sums the 4 partials per batch (the TP all-reduce done at
unshard time).

v3 design notes (vs the 204us baseline):
- P@V runs as (V^T P): matmul(yT[65,512], lhsT=v[:,65H:65H+65], rhs=pt) so y
  lands directly in the transposed layout the wo projection needs. This kills
  the 544 tiny N=65 matmuls (LDWEIGHTS-bound), all 32 PE transposes and the
  y_all->yt copies. The ones-column in V makes psum row 64 the softmax
  denominator.
- softmax normalization: reciprocal_approx_fast (single DVE op) of yT row 64,
  broadcast across partitions with K=1 f32r PE matmuls, copy to sbuf, one
  [64,512] DVE mul per head writes normalized yt. The normalize for a
  head-pair is emitted one score-step into the NEXT head-pair so the PE
  never idles on it (and the yT bank reuse stays deadlock-free).
- causal diag masking via affine_select zeroing on pt (post-exp, Pool
  engine) instead of -1e9 PE matmuls; exp issued per-head on diag tiles to
  skip the dead columns.
- RoPE: cos-mul reads the fp16 sbuf copy (raw) so the qkv psum's only
  reader is the scalar copy; qkv m-chains interleave with v-proj chains so
  psum drains hide under PE work. PSUM plan (8 banks): ps 1 + psv 1 +
  big{scores,perm,bcast,wo} 4 + yT 2.
- OUT is fp16 (halves output DMA; host accumulates partials in fp32).

Precision: fp16 matmuls everywhere; exp and softmax accumulation in fp32.
"""
import sys

sys.path.insert(0, "/opt/trn_rl_repo")

import numpy as np

import concourse.bass as bass
import concourse.mybir as mybir
import concourse.tile as tile
from concourse import bacc, bass_utils

B, L, D = 2, 2048, 1024
NH, HD = 16, 64
NCORES = 8
HPC = 4            # heads per core
LQB = 512          # Lq block per S^T unit
NLQ = L // LQB     # 4
NLT = L // 128     # 16
KT = D // 128      # 8 contraction tiles for projections

F32 = mybir.dt.float32
F32R = mybir.dt.float32r
F16 = mybir.dt.float16
BF16 = mybir.dt.bfloat16
I16 = mybir.dt.int16

# Schraudolph fp16-bit exp: exp(s) ~= bitcast_f16(int16(EXP_A*s + EXP_B)).
# ~1.8% rms sawtooth error; applied to a third of the off-diagonal score
# tiles to take exp load off the Scalar engine (the attention bottleneck).
EXP_A = 1024.0 / float(np.log(2.0))
EXP_B = 15.0 * 1024.0 - 59.0

_cache = {}


def build_nc(debug=False):
    nc = bacc.Bacc("TRN2", target_bir_lowering=False, debug=False)

    XT = nc.dram_tensor("XT", [D, L], F16, kind="ExternalInput")
    WQKT = nc.dram_tensor("WQKT", [D, 512], F16, kind="ExternalInput")
    WVT = nc.dram_tensor("WVT", [D, 260], F16, kind="ExternalInput")
    WOT = nc.dram_tensor("WOT", [HPC * HD, D], F16, kind="ExternalInput")
    PERM = nc.dram_tensor("PERM", [128, 128], F16, kind="ExternalInput")
    COS = nc.dram_tensor("COS", [128, L], F16, kind="ExternalInput")
    SIN = nc.dram_tensor("SIN", [128, L], F32, kind="ExternalInput")
    SELR = nc.dram_tensor("SELR", [33, 128], F16, kind="ExternalInput")
    SUMZ = nc.dram_tensor("SUMZ", [33, 512], F16, kind="ExternalInput")
    OUT = nc.dram_tensor("OUT", [L, D], F16, kind="ExternalOutput")

    Exp = mybir.ActivationFunctionType.Exp

    with tile.TileContext(nc) as tc:
        with (
            tc.tile_pool(name="consts", bufs=1) as cpool,
            tc.tile_pool(name="weights", bufs=1) as wpool,
            tc.tile_pool(name="qkrot", bufs=1) as rotpool,
            tc.tile_pool(name="vsb", bufs=1) as vpool,
            tc.tile_pool(name="ytr", bufs=1) as ytpool,
            tc.tile_pool(name="xt", bufs=16) as xpool,
            tc.tile_pool(name="raws", bufs=3) as rawpool,
            tc.tile_pool(name="tmps", bufs=3) as tpool,
            tc.tile_pool(name="pts", bufs=4) as ptpool,
            tc.tile_pool(name="bcss", bufs=2) as bcspool,
            tc.tile_pool(name="outsb", bufs=3) as opool,
            tc.tile_pool(name="psP", bufs=1, space="PSUM") as pspool,
            tc.tile_pool(name="psV", bufs=1, space="PSUM") as vvpool,
            tc.tile_pool(name="psB", bufs=2, space="PSUM") as bigpool,
            tc.tile_pool(name="psY", bufs=1, space="PSUM") as ypool,
        ):
            # ---- static loads, ordered by first use so the PE can start
            # as soon as wqk + the first x chunk land ---------------------
            def load_xt_tile(j, k):
                xs = slice(j * LQB, (j + 1) * LQB)
                t = xpool.tile([128, LQB], F16, tag="xt", name="xt")
                if j == 0 and k == 0:
                    # first tile gates the first matmul: halve its latency
                    # by splitting across two DMA queues
                    nc.sync.dma_start(t[0:64, :], XT[0:64, xs])
                    nc.sync.dma_start(t[64:128, :], XT[64:128, xs])
                else:
                    nc.sync.dma_start(t[:], XT[k * 128:(k + 1) * 128, xs])
                return t

            def load_xt(j):
                return [load_xt_tile(j, k) for k in range(KT)]

            wqk_sb = []
            xt0 = []
            for k in range(KT):
                w = wpool.tile([128, 512], F16, tag=f"wqk{k}", name=f"wqk{k}")
                if k == 0:
                    nc.sync.dma_start(w[0:64, :], WQKT[0:64, :])
                    nc.sync.dma_start(w[64:128, :], WQKT[64:128, :])
                else:
                    nc.sync.dma_start(w[:], WQKT[k * 128:(k + 1) * 128, :])
                wqk_sb.append(w)
                xt0.append(load_xt_tile(0, k))

            wvt_sb = []
            for k in range(KT):
                wv = wpool.tile([128, 260], F16, tag=f"wv{k}", name=f"wv{k}")
                nc.sync.dma_start(wv[:], WVT[k * 128:(k + 1) * 128, :])
                wvt_sb.append(wv)
            cos_sb = cpool.tile([128, L], F16, tag="cos")
            nc.sync.dma_start(cos_sb[:], COS[:, :])
            sin_sb = cpool.tile([128, L], F32, tag="sin")
            nc.sync.dma_start(sin_sb[:], SIN[:, :])
            perm_sb = cpool.tile([128, 128], F16, tag="perm")
            nc.sync.dma_start(perm_sb[:], PERM[:, :])
            wot_sb = []
            for c2 in range(2):
                w = wpool.tile([128, D], F16, tag=f"wo{c2}", name=f"wo{c2}")
                nc.sync.dma_start(w[:], WOT[c2 * 128:(c2 + 1) * 128, :])
                wot_sb.append(w)
            # selector: one K=33 matmul broadcasts sums row 0 across output
            # partitions 0-63 and row 32 across 64-127
            sel_sb = cpool.tile([33, 128], F16, tag="sel")
            nc.sync.dma_start(sel_sb[:], SELR[:, :])
            # persistent rowsum row-pair tiles (rows 0 and 32 are written;
            # the zeroed rest keeps the K=33 matmul NaN-free)
            sums_sb = []
            for i in range(2):
                s = cpool.tile([33, 512], F16, tag=f"sums{i}")
                nc.sync.dma_start(s[:], SUMZ[:, :])
                sums_sb.append(s)

            # persistent activation storage
            # qk_rot[m]: m=0,1 -> q head-pairs (h01, h23); m=2,3 -> k pairs
            qk_rot = [rotpool.tile([128, L], F16, tag=f"rot{m}", name=f"rot{m}")
                      for m in range(4)]
            v_sb = [vpool.tile([128, 260], F16, tag=f"v{t}", name=f"v{t}")
                    for t in range(NLT)]
            # yt_sb[hp]: transposed, normalized y for head-pair hp
            yt_sb = [ytpool.tile([128, L], F16, tag=f"yt{c2}", name=f"yt{c2}")
                     for c2 in range(2)]

            def qkv_chunk(j, xt):
                xs = slice(j * LQB, (j + 1) * LQB)
                # q/k head-pair tiles with rope, interleaved with the v
                # tiles so each psum's drain hides under the next PE chain
                for m in range(4):
                    ps = pspool.tile([128, 512], F32, tag="ps", name="ps")
                    for k in range(KT):
                        nc.tensor.matmul(
                            ps[:], wqk_sb[k][:, m * 128:(m + 1) * 128],
                            xt[k][:],
                            start=(k == 0), stop=(k == KT - 1),
                        )
                    raw = rawpool.tile([128, LQB], F16, tag="raw")
                    nc.scalar.copy(raw[:], ps[:])
                    # t1 reads the sbuf fp16 copy, so ps's only reader is
                    # the scalar copy — the next chain isn't gated on DVE
                    t1 = tpool.tile([128, LQB], F16, tag="t1")
                    nc.vector.tensor_mul(t1[:], raw[:], cos_sb[:, xs])
                    # v tile (natural L x hd layout, ones col after each
                    # head) — emitted between the m-chain and the perm
                    # matmul so the PE never head-blocks on the raw copy
                    ti = j * 4 + m
                    psv = vvpool.tile([128, 512], F32, tag="vv", name="vv")
                    for k in range(KT):
                        nc.tensor.matmul(
                            psv[:, 0:260], xt[k][:, m * 128:(m + 1) * 128],
                            wvt_sb[k][:],
                            start=(k == 0), stop=(k == KT - 1),
                        )
                    nc.scalar.copy(v_sb[ti][:], psv[:, 0:260])
                    nc.vector.memset(v_sb[ti][:, 64:260:65], 1.0)
                    pswt = bigpool.tile([128, 1024], F32, tag="big",
                                        name="psw")
                    psw = pswt[:, 0:512]
                    nc.tensor.matmul(psw, perm_sb[:], raw[:],
                                     start=True, stop=True)
                    t2 = tpool.tile([128, LQB], F16, tag="t2")
                    nc.vector.tensor_mul(t2[:], psw, sin_sb[:, xs])
                    nc.vector.tensor_add(qk_rot[m][:, xs], t1[:], t2[:])

            wo_ready = []

            def wo_tile(i):
                po = bigpool.tile([128, 1024], F32, tag="big", name="po")
                for half in range(2):
                    for c2 in range(2):
                        nc.tensor.matmul(
                            po[:, 512 * half:512 * half + 512],
                            yt_sb[c2][:, 128 * i:128 * i + 128],
                            wot_sb[c2][:, 512 * half:512 * half + 512],
                            start=(c2 == 0), stop=(c2 == 1),
                        )
                ob = opool.tile([128, 1024], F16, tag="ob")
                if i % 2 == 0:
                    nc.scalar.copy(ob[:], po[:])
                else:
                    nc.vector.tensor_copy(ob[:], po[:])
                nc.gpsimd.dma_start(OUT[128 * i:128 * i + 128, :], ob[:])

            def emit_wo(nmax):
                for _ in range(nmax):
                    if not wo_ready:
                        return
                    wo_tile(wo_ready.pop(0))

            def emit_norm(pending):
                """PE broadcast + sbuf copy + DVE muls for a finished
                head-pair; emitted behind other PE work so it never
                head-blocks the queue. Once a jq's second head-pair is
                normalized, its wo tiles become emittable."""
                hp, jq, yT, sums = pending
                # broadcast both heads' rowsums across partitions with one
                # K=33 matmul, then one approx-reciprocal covers both heads
                bc = vvpool.tile([128, 512], F32, tag="vv", name="bc")
                nc.tensor.matmul(bc[:], sel_sb[:], sums[:],
                                 start=True, stop=True)
                bcs = bcspool.tile([128, 512], F32, tag="bcs")
                nc.vector.reciprocal_approx_fast(bcs[:], bc[:])
                for h in range(2):
                    nc.vector.tensor_mul(
                        yt_sb[hp][64 * h:64 * h + 64,
                                  jq * LQB:(jq + 1) * LQB],
                        yT[h][0:64, :],
                        bcs[64 * h:64 * h + 64, :],
                    )
                if hp == 1:
                    wo_ready.extend(range(4 * jq, 4 * jq + 4))

            pending_norm = [None]

            def attention_jq(jq):
                nt = 4 * jq + 4  # causal: Lk tiles 0 .. 4jq+3
                for hp in range(2):
                    if hp == 1:
                        # fill the head-pair seam (PE waits on the previous
                        # pair's normalize chain anyway) with ready wo tiles
                        emit_wo(2 if jq < 3 else 4)
                    # each yT tile owns one PSUM bank; its t=0 matmul covers
                    # the full [0:512] width, so start=True zeroes the bank
                    # (no sibling chains share it)
                    yT = [ypool.tile([65, 512], F32, tag=f"yT{h}",
                                     name=f"yT{h}", bufs=1)
                          for h in range(2)]

                    def pv_th(t, pt, h):
                        off = max(0, t * 128 - jq * LQB)
                        H = 2 * hp + h
                        nc.tensor.matmul(
                            yT[h][:, off:512],
                            v_sb[t][:, 65 * H:65 * H + 65],
                            pt[:, 512 * h + off:512 * h + 512],
                            start=(t == 0), stop=(t == nt - 1),
                            skip_group_check=True,
                        )

                    prev = None
                    for t in range(nt):
                        diag = t >= 4 * jq
                        # causal trim: cols < off are fully masked
                        off = max(0, t * 128 - jq * LQB)
                        sp = bigpool.tile([128, 1024], F32, tag="big",
                                          name="sp")
                        for h in range(2):
                            hs = slice(64 * h, 64 * h + 64)
                            nc.tensor.matmul(
                                sp[:, 512 * h + off:512 * h + 512],
                                qk_rot[2 + hp][hs, t * 128:(t + 1) * 128],
                                qk_rot[hp][hs, jq * LQB + off:
                                           (jq + 1) * LQB],
                                start=True, stop=True,
                            )
                        # P@V lags the scores by one tile so the exp
                        # latency hides behind the next score matmuls
                        if prev is not None:
                            pv_th(*prev, 0)
                            pv_th(*prev, 1)
                        if t == 0 and pending_norm[0] is not None:
                            emit_norm(pending_norm[0])
                            pending_norm[0] = None
                        pt = ptpool.tile([128, 1024], F16, tag="pt")
                        if diag:
                            for h in range(2):
                                nc.scalar.activation(
                                    pt[:, 512 * h + off:512 * h + 512],
                                    sp[:, 512 * h + off:512 * h + 512],
                                    Exp,
                                )
                            # zero the upper triangle of the diag stripe
                            # (key > query) so P@V sees true zeros
                            for h in range(2):
                                nc.gpsimd.affine_select(
                                    out=pt[:, 512 * h + off:
                                           512 * h + off + 128],
                                    in_=pt[:, 512 * h + off:
                                           512 * h + off + 128],
                                    compare_op=mybir.AluOpType.is_ge,
                                    fill=0.0,
                                    base=0,
                                    pattern=[[1, 128]],
                                    channel_multiplier=-1,
                                )
                        elif t % 3 == 1:
                            # approximate exp on the DVE to unload Scalar
                            nc.vector.tensor_scalar(
                                pt[:, 0:1024].bitcast(I16),
                                sp[:, 0:1024],
                                EXP_A, EXP_B,
                                mybir.AluOpType.mult, mybir.AluOpType.add,
                            )
                        else:
                            nc.scalar.activation(pt[:, 0:1024], sp[:, 0:1024],
                                                 Exp)
                        prev = (t, pt)
                    pv_th(*prev, 0)
                    pv_th(*prev, 1)
                    # rowsums to sbuf right away (split engines so both
                    # copies run in parallel); the rest of the normalize
                    # is deferred into the next PE block
                    sums = sums_sb[hp]
                    nc.scalar.copy(sums[0:1, :], yT[0][64:65, :])
                    nc.vector.tensor_copy(sums[32:33, :], yT[1][64:65, :])
                    if pending_norm[0] is not None:
                        emit_norm(pending_norm[0])
                    pending_norm[0] = (hp, jq, yT, sums)

            # software pipeline: qkv runs one chunk ahead of attention so
            # the PE never waits on rope at the seams; wo tiles trail,
            # filling head-pair seams and chunk boundaries
            qkv_chunk(0, xt0)
            attention_jq(0)
            qkv_chunk(1, load_xt(1))
            attention_jq(1)
            qkv_chunk(2, load_xt(2))
            emit_wo(2)
            attention_jq(2)
            qkv_chunk(3, load_xt(3))
            emit_wo(2)
            attention_jq(3)
            emit_norm(pending_norm[0])
            pending_norm[0] = None
            emit_wo(16)

    nc.finalize()
    return nc


def prep_inputs(x, wqkv, wo):
    """Build the 8 per-core input dicts from the full-problem inputs."""
    x = np.asarray(x, dtype=np.float32)
    wqkv = np.asarray(wqkv, dtype=np.float32)
    wo = np.asarray(wo, dtype=np.float32)

    # rope tables
    inv_freq = 1.0 / (10000.0 ** (np.arange(0, HD, 2, dtype=np.float32) / HD))
    t = np.arange(L, dtype=np.float32)
    freqs = np.outer(t, inv_freq)                  # (L, 32)
    cos32 = np.cos(freqs).T.astype(np.float32)     # (32, L)
    sin32 = np.sin(freqs).T.astype(np.float32)
    COS = np.ascontiguousarray(np.tile(cos32, (4, 1)))           # (128, L)
    SIN = np.ascontiguousarray(
        np.concatenate([-sin32, sin32, -sin32, sin32], axis=0)
    )

    # 32-block swap permutation (within each head's 64 rows)
    PERM = np.zeros((128, 128), dtype=np.float32)
    for blk in range(2):
        o = 64 * blk
        PERM[o:o + 32, o + 32:o + 64] = np.eye(32)
        PERM[o + 32:o + 64, o:o + 32] = np.eye(32)

    # rowsum-broadcast selector: row 0 -> out partitions 0-63,
    # row 32 -> out partitions 64-127
    SEL = np.zeros((33, 128), dtype=np.float32)
    SEL[0, 0:64] = 1.0
    SEL[32, 64:128] = 1.0

    in_maps = []
    scale = np.float32(HD ** -0.5)
    for c in range(NCORES):
        b, g = divmod(c, 4)
        qrows = slice(256 * g, 256 * g + 256)
        krows = slice(1024 + 256 * g, 1024 + 256 * g + 256)
        vrows = slice(2048 + 256 * g, 2048 + 256 * g + 256)

        XT = np.ascontiguousarray(x[b].T)                        # (1024, 2048)
        wq = (wqkv[qrows, :] * scale).T                          # (1024, 256)
        wk = wqkv[krows, :].T
        WQKT = np.ascontiguousarray(np.concatenate([wq, wk], axis=1))
        vpart = wqkv[vrows, :].T                                 # (1024, 256)
        WVT = np.zeros((D, 260), dtype=np.float32)
        for h in range(HPC):
            WVT[:, 65 * h:65 * h + 64] = vpart[:, 64 * h:64 * h + 64]
        WOT = np.ascontiguousarray(wo[:, 256 * g:256 * g + 256].T)

        in_maps.append({
            "XT": XT.astype(np.float16),
            "WQKT": WQKT.astype(np.float16),
            "WVT": WVT.astype(np.float16),
            "WOT": WOT.astype(np.float16),
            "COS": COS.astype(np.float16),
            "SIN": SIN,
            "PERM": PERM.astype(np.float16),
            "SELR": SEL.astype(np.float16),
            "SUMZ": np.zeros((33, 512), dtype=np.float16),
        })
    return in_maps


def kernel(x, wqkv, wo):
    if "nc" not in _cache:
        _cache["nc"] = build_nc()
    nc = _cache["nc"]
    in_maps = prep_inputs(x, wqkv, wo)
    res = bass_utils.run_bass_kernel_spmd(nc, in_maps, list(range(NCORES)))
    outs = [res.results[c]["OUT"].astype(np.float32) for c in range(NCORES)]
    out0 = outs[0] + outs[1] + outs[2] + outs[3]
    out1 = outs[4] + outs[5] + outs[6] + outs[7]
    return np.stack([out0, out1]).astype(np.float32)
